# revision 17
# baseline (speedup 1.0000x reference)
"""CP tensor-regression-layer kernel for Trainium2 (8 NeuronCores).

Computation (matches the reference einsum pair):
    t[b, r]  = sum_{i,j,k} x[b,i,j,k] * f0[i,r] * f1[j,r] * f2[k,r]
    out[b, c] = sum_r t[b,r] * weight[r] * f3[c,r] + bias[0]

Strategy: data-parallel over the batch dim (32 batches per core, CP
factors replicated).  Per core the big contraction is restructured as
    z[r, b, k] = sum_{ij} (f0[i,r]*f1[j,r]) * x[b, ij, k]
which is a K=2304 matmul against the Khatri-Rao product KR of f0 and
f1, run as 18 K-chunks of 128 partitions in bf16 (rel-err budget 2e-2
comfortably admits bf16 inputs; fp32 PSUM accumulation).  The
k-contraction against f2 runs on the vector engine in fp32 from PSUM,
and the class projection against (weight*f3)^T is a pair of small
accumulating matmuls.

Host prep is layout-only: x is permuted so every DMA lands as 128
partitions x 6 KiB contiguous rows (the DMA engine's sweet spot --
2-3 KiB rows degrade to a packet trickle), and f0/f1 are index-
expanded (gather, no flops) into the same ij = 128*m+p layout so KR
is one on-device elementwise multiply -- no PE transposes and no
identity matrix, which otherwise sit on the critical path.

The kernel is HBM-bandwidth bound on loading x (~7.1 MB/core in bf16
at ~435 GB/s shared over two hardware DGE queues).  All DMAs are
issued up-front; the gpsimd queue is software DGE (slow) and carries
only small constants that nothing stream-critical waits on.  The tail
after the last byte is: one double-chunk of matmuls -> full-width
k-contract on DVE -> the second half of the class projection
accumulated into PSUM (the first half was accumulated mid-stream) ->
bias -> store.
"""

import os

import numpy as np

_B, _M1, _M2, _M3, _C, _R = 256, 48, 48, 48, 1000, 64
_NCORES = 8
_BL = _B // _NCORES          # 32 batches per core
_IJ = _M1 * _M2              # 2304 contraction size (i,j fused)
_NCH = _IJ // 128            # 18 K-chunks of 128 partitions
_KB = _BL * _M3              # 1536 moving columns (b,k fused)
_SL = 512                    # matmul slice width (one PSUM bank, fp32)

# DMA units in expected-arrival order: doubles alternate scalar/sync
# (sync's first item is the f01 factor table so its doubles lag half a
# slot); chunks 16,17 ride as trailing singles, 17 landing last.
_CHUNKS = [0, 1, 2, 3, 4, 5, 6, 7, 8, 9, 10, 11, 12, 13, 14, 15, 16, 17]
_ZA_N = 8                    # first 8 arriving chunks -> za, rest -> zb

_cache = {}


def _split_excess_waits(nc, mybir, max_waits=1):
    """Walrus in this container rejects >1 sync-wait per instruction
    ("Too many sync wait commands").  Move excess waits onto chained
    NoOps inserted just before the offending instruction (same engine,
    so program order preserves the gating)."""
    for bb in nc.m.functions[0].blocks:
        insts = bb.instructions
        i = 0
        while i < len(insts):
            inst = insts[i]
            si = getattr(inst, "sync_info", None)
            waits = list(si.on_wait) if si is not None and si.on_wait else []
            if len(waits) > max_waits:
                rest, keep = waits[:-max_waits], waits[-max_waits:]
                pos = i
                for j in range(0, len(rest), max_waits):
                    nop = mybir.InstNoOp(
                        name=f"I-waitsplit-{nc.next_id()}",
                        engine=inst.engine,
                        ins=[],
                        outs=[],
                        sync_info=mybir.SyncInfo(
                            on_wait=list(rest[j : j + max_waits]), on_update=[]
                        ),
                    )
                    nc.register_instruction(nop)
                    insts.insert(pos, nop)
                    pos += 1
                    i += 1
                si.on_wait = keep
            i += 1


def _bcast(ap, bass, shape3):
    """AP broadcast helper: make a 3D view with a stride-0 middle dim."""
    try:
        return ap.unsqueeze(1).broadcast_to(shape3)
    except Exception:
        a = ap.ap
        return bass.AP(
            tensor=ap.tensor,
            offset=ap.offset,
            ap=[list(a[0]), [0, shape3[1]], list(a[1])],
        )


def _build_program():
    import concourse.bass as bass
    import concourse.tile as tile
    from concourse import mybir

    f32 = mybir.dt.float32
    bf16 = mybir.dt.bfloat16

    nc = bass.Bass("TRN2", target_bir_lowering=False, debug=False,
                   num_devices=_NCORES)

    x_d = nc.dram_tensor("x", [128, _NCH, _BL, _M3], bf16, kind="ExternalInput")
    f01_d = nc.dram_tensor("f01", [128, _NCH, 2, _R], bf16,
                           kind="ExternalInput")
    f2t_d = nc.dram_tensor("f2t", [_R, _M3], f32, kind="ExternalInput")
    f3t_d = nc.dram_tensor("f3t", [_R, _C], f32, kind="ExternalInput")
    w_d = nc.dram_tensor("w", [_R, 1], f32, kind="ExternalInput")
    b_d = nc.dram_tensor("b", [1, 1], f32, kind="ExternalInput")
    out_d = nc.dram_tensor("out", [_BL, _C], f32, kind="ExternalOutput")

    with tile.TileContext(nc) as tc:
        with (
            tc.tile_pool(name="sb", bufs=1) as consts,
            tc.tile_pool(name="pz", bufs=1, space=bass.MemorySpace.PSUM) as pz,
        ):
            xp = work = consts
            # ---- ACT-table warm source: no DMA dependency ----
            wsrc = consts.tile([1, 1], f32)
            nc.gpsimd.memset(wsrc[:], 0.0)

            # ---- the KR factor table leads the sync queue (it gates all
            # matmuls); small consts ride the slow gpsimd software queue,
            # f2t first (needed earliest, for the za k-contract) ----
            f01 = consts.tile([128, _NCH, 2, _R], bf16)
            nc.sync.dma_start(out=f01[0:64], in_=f01_d[0:64])
            nc.scalar.dma_start(out=f01[64:128], in_=f01_d[64:128])

            # ---- x stream (bf16): nine double-chunk DMAs (6 KiB rows),
            # alternating across the two hardware DGE queues; the last
            # double (chunks 16,17) trails the stream on scalar.  The
            # small consts slot into the scalar queue between doubles
            # (the gpsimd software-DGE queue steals bandwidth and burns
            # the Pool engine in DRAIN, so it carries nothing). ----
            xfs = [None] * _NCH   # flat [128, _KB] views per chunk

            def issue_double(eng, n):
                m = 2 * n
                xt = xp.tile([128, 2, _KB], bf16, tag=f"xd{n}",
                             name=f"xd{n}")
                src = x_d[:, m : m + 2].rearrange("p m b k -> p m (b k)")
                eng.dma_start(out=xt[:], in_=src)
                xfs[m] = xt[:, 0]
                xfs[m + 1] = xt[:, 1]

            def issue_single(eng, m):
                xt = xp.tile([128, _KB], bf16, tag=f"xs{m}", name=f"xs{m}")
                eng.dma_start(out=xt[:],
                              in_=x_d[:, m].rearrange("p b k -> p (b k)"))
                xfs[m] = xt[:]

            issue_double(nc.sync, 0)
            f2t = consts.tile([_R, _M3], f32)
            nc.scalar.dma_start(out=f2t[:], in_=f2t_d[:])
            wsb = consts.tile([_R, 1], f32)
            nc.scalar.dma_start(out=wsb[:], in_=w_d[:])
            issue_double(nc.scalar, 1)
            issue_double(nc.sync, 2)
            bsb = consts.tile([_BL, 1], f32)
            b_ap = b_d[:]
            nc.scalar.dma_start(
                out=bsb[:],
                in_=bass.AP(tensor=b_ap.tensor, offset=b_ap.offset,
                            ap=[[0, _BL], [0, 1]]),
            )
            f3t = consts.tile([_R, _C], f32)
            nc.scalar.dma_start(out=f3t[:], in_=f3t_d[:])
            issue_double(nc.scalar, 3)
            issue_double(nc.sync, 4)
            issue_double(nc.scalar, 5)
            issue_double(nc.sync, 6)
            issue_double(nc.scalar, 7)
            issue_single(nc.sync, 16)
            issue_single(nc.scalar, 17)

            # touch the ACT Identity table now so the tail bias-adds don't
            # pay the on-demand ACT_TABLE_LOAD (~1.3us)
            warm = consts.tile([1, 1], f32)
            nc.scalar.add(warm[:], wsrc[:], 0.0)

            # ---- KR = f0 (x) f1 in [p, m, r] layout: one DVE multiply of
            # the host-gathered factor table ----
            kr = consts.tile([128, _NCH, _R], bf16)
            with nc.allow_low_precision(reason="bf16 within tolerance"):
                nc.vector.tensor_mul(kr[:], f01[:, :, 0, :], f01[:, :, 1, :])

            # weight folds into the class projection matrix (needed only
            # from the first proj accumulation, mid-stream)
            f3tw = consts.tile([_R, _C], bf16)
            with nc.allow_low_precision(reason="bf16 within tolerance"):
                nc.vector.tensor_scalar_mul(f3tw[:], f3t[:], wsb[:])

            # ---- main contraction: two PSUM accumulators ----
            za = pz.tile([_R, _KB], f32, tag="za")
            zb = pz.tile([_R, _KB], f32, tag="zb")

            def emit_chunk(m, ztile, start, stop):
                for s in range(_KB // _SL):
                    nc.tensor.matmul(
                        ztile[:, s * _SL : (s + 1) * _SL],
                        lhsT=kr[:, m, :],
                        rhs=xfs[m][:, s * _SL : (s + 1) * _SL],
                        start=start,
                        stop=stop,
                    )

            # k-contraction: zf = z * f2 (broadcast over b), reduce over k.
            # Free-axis reductions and PSUM reads are DVE-only; one
            # full-width mul + one reduce (instruction overhead beats any
            # finer split), final write directly in bf16 for the proj.
            def k_contract(ztile, t_r, zftag):
                z3 = ztile[:].rearrange("r (b k) -> r b k", k=_M3)
                zf = work.tile([_R, _BL, _M3], f32, tag=zftag, name=zftag)
                with nc.allow_low_precision(
                    reason="bf16 within tolerance"
                ):
                    nc.vector.tensor_mul(
                        zf[:], z3[:],
                        _bcast(f2t[:], bass, (_R, _BL, _M3)),
                    )
                    nc.vector.reduce_sum(
                        t_r[:], zf[:], axis=mybir.AxisListType.X
                    )

            with tc.tile_pool(
                name="po", bufs=1, space=bass.MemorySpace.PSUM
            ) as po:
                opx = po.tile([_BL, 2 * _SL], f32, tag="opx")

                def proj(t_r, start, stop):
                    nc.tensor.matmul(opx[:, :_SL], lhsT=t_r[:],
                                     rhs=f3tw[:, :_SL],
                                     start=start, stop=stop)
                    nc.tensor.matmul(opx[:, _SL:_C], lhsT=t_r[:],
                                     rhs=f3tw[:, _SL:],
                                     start=start, stop=stop)

                # za: first 10 chunks in expected-arrival order, with the
                # mid-stream contract + first proj accumulation
                za_c, zb_c = _CHUNKS[:_ZA_N], _CHUNKS[_ZA_N:]
                for i, m in enumerate(za_c):
                    emit_chunk(m, za, i == 0, i == len(za_c) - 1)
                ta_r = work.tile([_R, _BL], bf16, tag="ta")
                k_contract(za, ta_r, "zfa")

                # zb chunks; proj_a slots in after the second pair (PE is
                # waiting on ta_r / later arrivals around then anyway)
                for i, m in enumerate(zb_c[:4]):
                    emit_chunk(m, zb, i == 0, False)
                proj(ta_r, True, False)
                for m in zb_c[4:]:
                    emit_chunk(m, zb, False, m == zb_c[-1])

                # tail: contract zb, finish proj, bias + store
                tb_r = work.tile([_R, _BL], bf16, tag="tb")
                k_contract(zb, tb_r, "zfb")
                proj(tb_r, False, True)

                osb = work.tile([_BL, _C], f32, tag="osb")
                nc.scalar.add(osb[:], opx[:, :_C], bsb[:])
                nc.sync.dma_start(out=out_d[:], in_=osb[:])

    _split_excess_waits(nc, mybir)
    return nc


def _get_program():
    if "nc" not in _cache:
        _cache["nc"] = _build_program()
    return _cache["nc"]


def _host_prep(x, weight, f0, f1, f2, f3, bias):
    """Shard x over cores (batch dim) in a DMA-friendly layout and cast
    to bf16; index-expand f0/f1 into the same ij = 128*m+p layout
    (gather + transpose + dtype cast only -- no arithmetic)."""
    import ml_dtypes

    bfl = ml_dtypes.bfloat16
    x = np.ascontiguousarray(np.asarray(x, dtype=np.float32))
    ij = np.arange(_IJ)
    f01 = np.stack(
        [np.asarray(f0, np.float32)[ij // _M2],
         np.asarray(f1, np.float32)[ij % _M2]], axis=1
    )  # [ij, 2, R]
    f01x = np.ascontiguousarray(
        f01.reshape(_NCH, 128, 2, _R).transpose(1, 0, 2, 3).astype(bfl)
    )
    f2t = np.ascontiguousarray(np.asarray(f2, np.float32).T)
    f3t = np.ascontiguousarray(np.asarray(f3, np.float32).T)
    w = np.ascontiguousarray(np.asarray(weight, np.float32).reshape(_R, 1))
    b = np.ascontiguousarray(np.asarray(bias, np.float32).reshape(1, 1))
    in_maps = []
    for c in range(_NCORES):
        xc = x[c * _BL : (c + 1) * _BL]
        # [b, ij, k] -> [p, m, b, k] with ij = 128*m + p
        xd = np.ascontiguousarray(
            xc.reshape(_BL, _NCH, 128, _M3)
            .transpose(2, 1, 0, 3)
            .astype(bfl)
        )
        in_maps.append(
            {"x": xd, "f01": f01x, "f2t": f2t, "f3t": f3t, "w": w, "b": b}
        )
    return in_maps


LAST_EXEC_NS = None


def kernel(x, weight, f0, f1, f2, f3, bias):
    global LAST_EXEC_NS
    from concourse.bass_utils import run_bass_kernel_spmd

    nc = _get_program()
    in_maps = _host_prep(x, weight, f0, f1, f2, f3, bias)
    trace = bool(int(os.environ.get("BASS_KERNEL_TRACE", "0")))
    res = run_bass_kernel_spmd(nc, in_maps, list(range(_NCORES)), trace=trace)
    LAST_EXEC_NS = res.exec_time_ns
    out = np.concatenate([res.results[c]["out"] for c in range(_NCORES)], axis=0)
    return np.ascontiguousarray(out.astype(np.float32, copy=False))


# revision 18
# speedup vs baseline: 1.0960x; 1.0960x over previous
"""CP tensor-regression-layer kernel for Trainium2 (8 NeuronCores).

Computation (matches the reference einsum pair):
    t[b, r]  = sum_{i,j,k} x[b,i,j,k] * f0[i,r] * f1[j,r] * f2[k,r]
    out[b, c] = sum_r t[b,r] * weight[r] * f3[c,r] + bias[0]

Strategy: data-parallel over the batch dim (32 batches per core, CP
factors replicated).  Per core the big contraction is restructured as
    z[r, b, k] = sum_{ij} (f0[i,r]*f1[j,r]) * x[b, ij, k]
which is a K=2304 matmul against the Khatri-Rao product KR of f0 and
f1, run as 18 K-chunks of 128 partitions in bf16 (rel-err budget 2e-2
comfortably admits bf16 inputs; fp32 PSUM accumulation).  The
k-contraction against f2 runs on the vector engine in fp32 from PSUM,
and the class projection against (weight*f3)^T is a pair of small
accumulating matmuls.

Host prep is layout-only: x is permuted so every DMA lands as 128
partitions x 6 KiB contiguous rows (the DMA engine's sweet spot --
2-3 KiB rows degrade to a packet trickle), and f0/f1 are index-
expanded (gather, no flops) into the same ij = 128*m+p layout so KR
is one on-device elementwise multiply -- no PE transposes and no
identity matrix, which otherwise sit on the critical path.

The kernel is HBM-bandwidth bound on loading x (~7.1 MB/core in bf16
at ~435 GB/s shared over two hardware DGE queues).  All DMAs are
issued up-front; the gpsimd queue is software DGE (slow) and carries
only small constants that nothing stream-critical waits on.  The tail
after the last byte is: one double-chunk of matmuls -> full-width
k-contract on DVE -> the second half of the class projection
accumulated into PSUM (the first half was accumulated mid-stream) ->
bias -> store.
"""

import os

import numpy as np

_B, _M1, _M2, _M3, _C, _R = 256, 48, 48, 48, 1000, 64
_NCORES = 8
_BL = _B // _NCORES          # 32 batches per core
_IJ = _M1 * _M2              # 2304 contraction size (i,j fused)
_NCH = _IJ // 128            # 18 K-chunks of 128 partitions
_KB = _BL * _M3              # 1536 moving columns (b,k fused)
_SL = 512                    # matmul slice width (one PSUM bank, fp32)

# DMA units in expected-arrival order: doubles alternate scalar/sync
# (sync's first item is the f01 factor table so its doubles lag half a
# slot); chunks 16,17 ride as trailing singles, 17 landing last.
_CHUNKS = [0, 1, 2, 3, 4, 5, 6, 7, 8, 9, 10, 11, 12, 13, 14, 15, 16, 17]
_ZA_N = 8                    # first 8 arriving chunks -> za, rest -> zb

_cache = {}


def _split_excess_waits(nc, mybir, max_waits=1):
    """Walrus in this container rejects >1 sync-wait per instruction
    ("Too many sync wait commands").  Move excess waits onto chained
    NoOps inserted just before the offending instruction (same engine,
    so program order preserves the gating)."""
    for bb in nc.m.functions[0].blocks:
        insts = bb.instructions
        i = 0
        while i < len(insts):
            inst = insts[i]
            si = getattr(inst, "sync_info", None)
            waits = list(si.on_wait) if si is not None and si.on_wait else []
            if len(waits) > max_waits:
                rest, keep = waits[:-max_waits], waits[-max_waits:]
                pos = i
                for j in range(0, len(rest), max_waits):
                    nop = mybir.InstNoOp(
                        name=f"I-waitsplit-{nc.next_id()}",
                        engine=inst.engine,
                        ins=[],
                        outs=[],
                        sync_info=mybir.SyncInfo(
                            on_wait=list(rest[j : j + max_waits]), on_update=[]
                        ),
                    )
                    nc.register_instruction(nop)
                    insts.insert(pos, nop)
                    pos += 1
                    i += 1
                si.on_wait = keep
            i += 1


def _bcast(ap, bass, shape3):
    """AP broadcast helper: make a 3D view with a stride-0 middle dim."""
    try:
        return ap.unsqueeze(1).broadcast_to(shape3)
    except Exception:
        a = ap.ap
        return bass.AP(
            tensor=ap.tensor,
            offset=ap.offset,
            ap=[list(a[0]), [0, shape3[1]], list(a[1])],
        )


def _build_program():
    import concourse.bass as bass
    import concourse.tile as tile
    from concourse import mybir

    f32 = mybir.dt.float32
    bf16 = mybir.dt.bfloat16

    nc = bass.Bass("TRN2", target_bir_lowering=False, debug=False,
                   num_devices=_NCORES)

    x_d = nc.dram_tensor("x", [128, _NCH, _BL, _M3], bf16, kind="ExternalInput")
    f01_d = nc.dram_tensor("f01", [128, _NCH, 2, _R], bf16,
                           kind="ExternalInput")
    f2t_d = nc.dram_tensor("f2t", [_R, _M3], f32, kind="ExternalInput")
    f3t_d = nc.dram_tensor("f3t", [_R, _C], f32, kind="ExternalInput")
    w_d = nc.dram_tensor("w", [_R, 1], f32, kind="ExternalInput")
    b_d = nc.dram_tensor("b", [1, 1], f32, kind="ExternalInput")
    out_d = nc.dram_tensor("out", [_BL, _C], f32, kind="ExternalOutput")

    with tile.TileContext(nc) as tc:
        with (
            tc.tile_pool(name="consts", bufs=1) as consts,
            tc.tile_pool(name="xp", bufs=1) as xp,
            tc.tile_pool(name="work", bufs=1) as work,
            tc.tile_pool(name="pz", bufs=1, space=bass.MemorySpace.PSUM) as pz,
        ):
            # ---- ACT-table warm source: no DMA dependency ----
            wsrc = consts.tile([1, 1], f32)
            nc.gpsimd.memset(wsrc[:], 0.0)

            # ---- the KR factor table leads the sync queue (it gates all
            # matmuls); small consts ride the slow gpsimd software queue,
            # f2t first (needed earliest, for the za k-contract) ----
            f01 = consts.tile([128, _NCH, 2, _R], bf16)
            nc.sync.dma_start(out=f01[0:64], in_=f01_d[0:64])
            nc.scalar.dma_start(out=f01[64:128], in_=f01_d[64:128])

            # ---- x stream (bf16): nine double-chunk DMAs (6 KiB rows),
            # alternating across the two hardware DGE queues; the last
            # double (chunks 16,17) trails the stream on scalar.  The
            # small consts slot into the scalar queue between doubles
            # (the gpsimd software-DGE queue steals bandwidth and burns
            # the Pool engine in DRAIN, so it carries nothing). ----
            xfs = [None] * _NCH   # flat [128, _KB] views per chunk

            def issue_double(eng, n):
                m = 2 * n
                xt = xp.tile([128, 2, _KB], bf16, tag=f"xd{n}",
                             name=f"xd{n}")
                src = x_d[:, m : m + 2].rearrange("p m b k -> p m (b k)")
                eng.dma_start(out=xt[:], in_=src)
                xfs[m] = xt[:, 0]
                xfs[m + 1] = xt[:, 1]

            def issue_single(eng, m):
                xt = xp.tile([128, _KB], bf16, tag=f"xs{m}", name=f"xs{m}")
                eng.dma_start(out=xt[:],
                              in_=x_d[:, m].rearrange("p b k -> p (b k)"))
                xfs[m] = xt[:]

            issue_double(nc.sync, 0)
            f2t = consts.tile([_R, _M3], f32)
            nc.scalar.dma_start(out=f2t[:], in_=f2t_d[:])
            wsb = consts.tile([_R, 1], f32)
            nc.scalar.dma_start(out=wsb[:], in_=w_d[:])
            issue_double(nc.scalar, 1)
            issue_double(nc.sync, 2)
            bsb = consts.tile([_BL, 1], f32)
            b_ap = b_d[:]
            nc.scalar.dma_start(
                out=bsb[:],
                in_=bass.AP(tensor=b_ap.tensor, offset=b_ap.offset,
                            ap=[[0, _BL], [0, 1]]),
            )
            f3t = consts.tile([_R, _C], f32)
            nc.scalar.dma_start(out=f3t[:], in_=f3t_d[:])
            issue_double(nc.scalar, 3)
            issue_double(nc.sync, 4)
            issue_double(nc.scalar, 5)
            issue_double(nc.sync, 6)
            issue_double(nc.scalar, 7)
            issue_single(nc.sync, 16)
            issue_single(nc.scalar, 17)

            # touch the ACT Identity table now so the tail bias-adds don't
            # pay the on-demand ACT_TABLE_LOAD (~1.3us)
            warm = consts.tile([1, 1], f32)
            nc.scalar.add(warm[:], wsrc[:], 0.0)

            # ---- KR = f0 (x) f1 in [p, m, r] layout: one DVE multiply of
            # the host-gathered factor table ----
            kr = consts.tile([128, _NCH, _R], bf16)
            with nc.allow_low_precision(reason="bf16 within tolerance"):
                nc.vector.tensor_mul(kr[:], f01[:, :, 0, :], f01[:, :, 1, :])

            # weight folds into the class projection matrix (needed only
            # from the first proj accumulation, mid-stream)
            f3tw = consts.tile([_R, _C], bf16)
            with nc.allow_low_precision(reason="bf16 within tolerance"):
                nc.vector.tensor_scalar_mul(f3tw[:], f3t[:], wsb[:])

            # ---- main contraction: two PSUM accumulators ----
            za = pz.tile([_R, _KB], f32, tag="za")
            zb = pz.tile([_R, _KB], f32, tag="zb")

            def emit_chunk(m, ztile, start, stop):
                for s in range(_KB // _SL):
                    nc.tensor.matmul(
                        ztile[:, s * _SL : (s + 1) * _SL],
                        lhsT=kr[:, m, :],
                        rhs=xfs[m][:, s * _SL : (s + 1) * _SL],
                        start=start,
                        stop=stop,
                    )

            # k-contraction: zf = z * f2 (broadcast over b), reduce over k.
            # Free-axis reductions and PSUM reads are DVE-only; one
            # full-width mul + one reduce (instruction overhead beats any
            # finer split), final write directly in bf16 for the proj.
            def k_contract(ztile, t_r, zftag):
                z3 = ztile[:].rearrange("r (b k) -> r b k", k=_M3)
                zf = work.tile([_R, _BL, _M3], f32, tag=zftag, name=zftag)
                with nc.allow_low_precision(
                    reason="bf16 within tolerance"
                ):
                    nc.vector.tensor_mul(
                        zf[:], z3[:],
                        _bcast(f2t[:], bass, (_R, _BL, _M3)),
                    )
                    nc.vector.reduce_sum(
                        t_r[:], zf[:], axis=mybir.AxisListType.X
                    )

            with tc.tile_pool(
                name="po", bufs=1, space=bass.MemorySpace.PSUM
            ) as po:
                op0 = po.tile([_BL, _SL], f32, tag="op0")
                op1 = po.tile([_BL, _C - _SL], f32, tag="op1")

                def proj(t_r, start, stop):
                    nc.tensor.matmul(op0[:], lhsT=t_r[:],
                                     rhs=f3tw[:, :_SL],
                                     start=start, stop=stop)
                    nc.tensor.matmul(op1[:], lhsT=t_r[:],
                                     rhs=f3tw[:, _SL:],
                                     start=start, stop=stop)

                # za: first 10 chunks in expected-arrival order, with the
                # mid-stream contract + first proj accumulation
                za_c, zb_c = _CHUNKS[:_ZA_N], _CHUNKS[_ZA_N:]
                for i, m in enumerate(za_c):
                    emit_chunk(m, za, i == 0, i == len(za_c) - 1)
                ta_r = work.tile([_R, _BL], bf16, tag="ta")
                k_contract(za, ta_r, "zfa")

                # zb chunks; proj_a slots in after the second pair (PE is
                # waiting on ta_r / later arrivals around then anyway)
                for i, m in enumerate(zb_c[:4]):
                    emit_chunk(m, zb, i == 0, False)
                proj(ta_r, True, False)
                for m in zb_c[4:]:
                    emit_chunk(m, zb, False, m == zb_c[-1])

                # tail: contract zb, finish proj, bias + store
                tb_r = work.tile([_R, _BL], bf16, tag="tb")
                k_contract(zb, tb_r, "zfb")
                proj(tb_r, False, True)

                osb = work.tile([_BL, _C], f32, tag="osb")
                nc.scalar.add(osb[:, :_SL], op0[:], bsb[:])
                nc.sync.dma_start(out=out_d[:, :_SL], in_=osb[:, :_SL])
                nc.scalar.add(osb[:, _SL:], op1[:], bsb[:])
                nc.sync.dma_start(out=out_d[:, _SL:], in_=osb[:, _SL:])

    _split_excess_waits(nc, mybir)
    return nc


def _get_program():
    if "nc" not in _cache:
        _cache["nc"] = _build_program()
    return _cache["nc"]


def _host_prep(x, weight, f0, f1, f2, f3, bias):
    """Shard x over cores (batch dim) in a DMA-friendly layout and cast
    to bf16; index-expand f0/f1 into the same ij = 128*m+p layout
    (gather + transpose + dtype cast only -- no arithmetic)."""
    import ml_dtypes

    bfl = ml_dtypes.bfloat16
    x = np.ascontiguousarray(np.asarray(x, dtype=np.float32))
    ij = np.arange(_IJ)
    f01 = np.stack(
        [np.asarray(f0, np.float32)[ij // _M2],
         np.asarray(f1, np.float32)[ij % _M2]], axis=1
    )  # [ij, 2, R]
    f01x = np.ascontiguousarray(
        f01.reshape(_NCH, 128, 2, _R).transpose(1, 0, 2, 3).astype(bfl)
    )
    f2t = np.ascontiguousarray(np.asarray(f2, np.float32).T)
    f3t = np.ascontiguousarray(np.asarray(f3, np.float32).T)
    w = np.ascontiguousarray(np.asarray(weight, np.float32).reshape(_R, 1))
    b = np.ascontiguousarray(np.asarray(bias, np.float32).reshape(1, 1))
    in_maps = []
    for c in range(_NCORES):
        xc = x[c * _BL : (c + 1) * _BL]
        # [b, ij, k] -> [p, m, b, k] with ij = 128*m + p
        xd = np.ascontiguousarray(
            xc.reshape(_BL, _NCH, 128, _M3)
            .transpose(2, 1, 0, 3)
            .astype(bfl)
        )
        in_maps.append(
            {"x": xd, "f01": f01x, "f2t": f2t, "f3t": f3t, "w": w, "b": b}
        )
    return in_maps


LAST_EXEC_NS = None


def kernel(x, weight, f0, f1, f2, f3, bias):
    global LAST_EXEC_NS
    from concourse.bass_utils import run_bass_kernel_spmd

    nc = _get_program()
    in_maps = _host_prep(x, weight, f0, f1, f2, f3, bias)
    trace = bool(int(os.environ.get("BASS_KERNEL_TRACE", "0")))
    res = run_bass_kernel_spmd(nc, in_maps, list(range(_NCORES)), trace=trace)
    LAST_EXEC_NS = res.exec_time_ns
    out = np.concatenate([res.results[c]["out"] for c in range(_NCORES)], axis=0)
    return np.ascontiguousarray(out.astype(np.float32, copy=False))


# revision 19
# speedup vs baseline: 1.1309x; 1.0318x over previous
"""CP tensor-regression-layer kernel for Trainium2 (8 NeuronCores).

Computation (matches the reference einsum pair):
    t[b, r]  = sum_{i,j,k} x[b,i,j,k] * f0[i,r] * f1[j,r] * f2[k,r]
    out[b, c] = sum_r t[b,r] * weight[r] * f3[c,r] + bias[0]

Strategy: data-parallel over the batch dim (32 batches per core, CP
factors replicated).  Per core the big contraction is restructured as
    z[r, b, k] = sum_{ij} (f0[i,r]*f1[j,r]) * x[b, ij, k]
which is a K=2304 matmul against the Khatri-Rao product KR of f0 and
f1, run as 18 K-chunks of 128 partitions in bf16 (rel-err budget 2e-2
comfortably admits bf16 inputs; fp32 PSUM accumulation).  The
k-contraction against f2 runs on the vector engine in fp32 from PSUM,
and the class projection against (weight*f3)^T is a pair of small
accumulating matmuls.

Host prep is layout-only: x is permuted so every DMA lands as 128
partitions x 6 KiB contiguous rows (the DMA engine's sweet spot --
2-3 KiB rows degrade to a packet trickle), and f0/f1 are index-
expanded (gather, no flops) into the same ij = 128*m+p layout so KR
is one on-device elementwise multiply -- no PE transposes and no
identity matrix, which otherwise sit on the critical path.

The kernel is HBM-bandwidth bound on loading x (~7.1 MB/core in bf16
at ~435 GB/s shared over two hardware DGE queues; the gpsimd queue is
software DGE -- slow and Pool-engine-hungry -- and carries nothing).
All DMAs are issued up-front: the f01 factor table leads both queues
as partition halves, small consts slot between the first doubles on
the scalar queue, x rides as 8 double-chunk DMAs plus two trailing
singles so chunk 17 lands last.  The tail after the last byte is: the
final chunk's matmuls -> full-width k-contract on DVE -> the second
half of the class projection accumulated into PSUM (the first half
was accumulated mid-stream) -> bias -> store.
"""

import os

import numpy as np

_B, _M1, _M2, _M3, _C, _R = 256, 48, 48, 48, 1000, 64
_NCORES = 8
_BL = _B // _NCORES          # 32 batches per core
_IJ = _M1 * _M2              # 2304 contraction size (i,j fused)
_NCH = _IJ // 128            # 18 K-chunks of 128 partitions
_KB = _BL * _M3              # 1536 moving columns (b,k fused)
_SL = 512                    # matmul slice width (one PSUM bank, fp32)

# DMA units in expected-arrival order: doubles alternate scalar/sync
# (sync's first item is the f01 factor table so its doubles lag half a
# slot); chunks 16,17 ride as trailing singles, 17 landing last.
_CHUNKS = [0, 1, 2, 3, 4, 5, 6, 7, 8, 9, 10, 11, 12, 13, 14, 15, 16, 17]
_ZA_N = 8                    # first 8 arriving chunks -> za, rest -> zb

_cache = {}


def _split_excess_waits(nc, mybir, max_waits=1):
    """Walrus in this container rejects >1 sync-wait per instruction
    ("Too many sync wait commands").  Move excess waits onto chained
    NoOps inserted just before the offending instruction (same engine,
    so program order preserves the gating)."""
    for bb in nc.m.functions[0].blocks:
        insts = bb.instructions
        i = 0
        while i < len(insts):
            inst = insts[i]
            si = getattr(inst, "sync_info", None)
            waits = list(si.on_wait) if si is not None and si.on_wait else []
            if len(waits) > max_waits:
                rest, keep = waits[:-max_waits], waits[-max_waits:]
                pos = i
                for j in range(0, len(rest), max_waits):
                    nop = mybir.InstNoOp(
                        name=f"I-waitsplit-{nc.next_id()}",
                        engine=inst.engine,
                        ins=[],
                        outs=[],
                        sync_info=mybir.SyncInfo(
                            on_wait=list(rest[j : j + max_waits]), on_update=[]
                        ),
                    )
                    nc.register_instruction(nop)
                    insts.insert(pos, nop)
                    pos += 1
                    i += 1
                si.on_wait = keep
            i += 1


def _bcast(ap, bass, shape3):
    """AP broadcast helper: make a 3D view with a stride-0 middle dim."""
    try:
        return ap.unsqueeze(1).broadcast_to(shape3)
    except Exception:
        a = ap.ap
        return bass.AP(
            tensor=ap.tensor,
            offset=ap.offset,
            ap=[list(a[0]), [0, shape3[1]], list(a[1])],
        )


def _build_program():
    import concourse.bass as bass
    import concourse.tile as tile
    from concourse import mybir

    f32 = mybir.dt.float32
    bf16 = mybir.dt.bfloat16

    nc = bass.Bass("TRN2", target_bir_lowering=False, debug=False,
                   num_devices=_NCORES)

    x_d = nc.dram_tensor("x", [128, _NCH, _BL, _M3], bf16, kind="ExternalInput")
    f01_d = nc.dram_tensor("f01", [128, _NCH, 2, _R], bf16,
                           kind="ExternalInput")
    f2t_d = nc.dram_tensor("f2t", [_R, _M3], f32, kind="ExternalInput")
    f3t_d = nc.dram_tensor("f3t", [_R, _C], f32, kind="ExternalInput")
    w_d = nc.dram_tensor("w", [_R, 1], f32, kind="ExternalInput")
    b_d = nc.dram_tensor("b", [1, 1], f32, kind="ExternalInput")
    out_d = nc.dram_tensor("out", [_BL, _C], f32, kind="ExternalOutput")

    with tile.TileContext(nc) as tc:
        with (
            tc.tile_pool(name="consts", bufs=1) as consts,
            tc.tile_pool(name="xp", bufs=1) as xp,
            tc.tile_pool(name="work", bufs=1) as work,
            tc.tile_pool(name="pz", bufs=1, space=bass.MemorySpace.PSUM) as pz,
        ):
            # ---- ACT-table warm source: no DMA dependency ----
            wsrc = consts.tile([1, 1], f32)
            nc.gpsimd.memset(wsrc[:], 0.0)

            # ---- the KR factor table leads both hardware queues as
            # partition halves (it gates all matmuls) ----
            f01 = consts.tile([128, _NCH, 2, _R], bf16)
            nc.sync.dma_start(out=f01[0:64], in_=f01_d[0:64])
            nc.scalar.dma_start(out=f01[64:128], in_=f01_d[64:128])

            # ---- x stream (bf16): eight double-chunk DMAs (6 KiB rows)
            # plus two trailing singles, alternating across the two
            # hardware DGE queues so chunk 17 lands last.  The small
            # consts slot into the scalar queue between doubles (the
            # gpsimd software-DGE queue steals bandwidth and burns the
            # Pool engine in DRAIN, so it carries nothing). ----
            xfs = [None] * _NCH   # flat [128, _KB] views per chunk

            def issue_double(eng, n):
                m = 2 * n
                xt = xp.tile([128, 2, _KB], bf16, tag=f"xd{n}",
                             name=f"xd{n}")
                src = x_d[:, m : m + 2].rearrange("p m b k -> p m (b k)")
                eng.dma_start(out=xt[:], in_=src)
                xfs[m] = xt[:, 0]
                xfs[m + 1] = xt[:, 1]

            def issue_single(eng, m):
                xt = xp.tile([128, _KB], bf16, tag=f"xs{m}", name=f"xs{m}")
                eng.dma_start(out=xt[:],
                              in_=x_d[:, m].rearrange("p b k -> p (b k)"))
                xfs[m] = xt[:]

            issue_double(nc.sync, 0)
            f2t = consts.tile([_R, _M3], f32)
            nc.scalar.dma_start(out=f2t[:], in_=f2t_d[:])
            wsb = consts.tile([_R, 1], f32)
            nc.scalar.dma_start(out=wsb[:], in_=w_d[:])
            issue_double(nc.scalar, 1)
            issue_double(nc.sync, 2)
            bsb = consts.tile([_BL, 1], f32)
            b_ap = b_d[:]
            nc.scalar.dma_start(
                out=bsb[:],
                in_=bass.AP(tensor=b_ap.tensor, offset=b_ap.offset,
                            ap=[[0, _BL], [0, 1]]),
            )
            f3t = consts.tile([_R, _C], f32)
            nc.scalar.dma_start(out=f3t[:], in_=f3t_d[:])
            issue_double(nc.scalar, 3)
            issue_double(nc.sync, 4)
            issue_double(nc.scalar, 5)
            issue_double(nc.sync, 6)
            issue_double(nc.scalar, 7)
            issue_single(nc.sync, 16)
            issue_single(nc.scalar, 17)

            # touch the ACT Identity table now so the tail bias-adds don't
            # pay the on-demand ACT_TABLE_LOAD (~1.3us)
            warm = consts.tile([1, 1], f32)
            nc.scalar.add(warm[:], wsrc[:], 0.0)

            # ---- KR = f0 (x) f1 in [p, m, r] layout: one DVE multiply of
            # the host-gathered factor table ----
            kr = consts.tile([128, _NCH, _R], bf16)
            with nc.allow_low_precision(reason="bf16 within tolerance"):
                nc.vector.tensor_mul(kr[:], f01[:, :, 0, :], f01[:, :, 1, :])

            # weight folds into the class projection matrix (needed only
            # from the first proj accumulation, mid-stream)
            f3tw = consts.tile([_R, _C], bf16)
            with nc.allow_low_precision(reason="bf16 within tolerance"):
                nc.vector.tensor_scalar_mul(f3tw[:], f3t[:], wsb[:])

            # ---- main contraction: two PSUM accumulators ----
            za = pz.tile([_R, _KB], f32, tag="za")
            zb = pz.tile([_R, _KB], f32, tag="zb")

            def emit_chunk(m, ztile, start, stop):
                for s in range(_KB // _SL):
                    nc.tensor.matmul(
                        ztile[:, s * _SL : (s + 1) * _SL],
                        lhsT=kr[:, m, :],
                        rhs=xfs[m][:, s * _SL : (s + 1) * _SL],
                        start=start,
                        stop=stop,
                    )

            # k-contraction: zf = z * f2 (broadcast over b), reduce over k.
            # Free-axis reductions and PSUM reads are DVE-only; one
            # full-width mul + one reduce (instruction overhead beats any
            # finer split), final write directly in bf16 for the proj.
            def k_contract(ztile, t_r, zftag):
                z3 = ztile[:].rearrange("r (b k) -> r b k", k=_M3)
                zf = work.tile([_R, _BL, _M3], f32, tag=zftag, name=zftag)
                with nc.allow_low_precision(
                    reason="bf16 within tolerance"
                ):
                    nc.vector.tensor_mul(
                        zf[:], z3[:],
                        _bcast(f2t[:], bass, (_R, _BL, _M3)),
                    )
                    nc.vector.reduce_sum(
                        t_r[:], zf[:], axis=mybir.AxisListType.X
                    )

            with tc.tile_pool(
                name="po", bufs=1, space=bass.MemorySpace.PSUM
            ) as po:
                op0 = po.tile([_BL, _SL], f32, tag="op0")
                op1 = po.tile([_BL, _C - _SL], f32, tag="op1")

                def proj(t_r, start, stop):
                    nc.tensor.matmul(op0[:], lhsT=t_r[:],
                                     rhs=f3tw[:, :_SL],
                                     start=start, stop=stop)
                    nc.tensor.matmul(op1[:], lhsT=t_r[:],
                                     rhs=f3tw[:, _SL:],
                                     start=start, stop=stop)

                # za: first 10 chunks in expected-arrival order, with the
                # mid-stream contract + first proj accumulation
                za_c, zb_c = _CHUNKS[:_ZA_N], _CHUNKS[_ZA_N:]
                for i, m in enumerate(za_c):
                    emit_chunk(m, za, i == 0, i == len(za_c) - 1)
                ta_r = work.tile([_R, _BL], bf16, tag="ta")
                k_contract(za, ta_r, "zfa")

                # zb chunks; proj_a slots in after the second pair (PE is
                # waiting on ta_r / later arrivals around then anyway)
                for i, m in enumerate(zb_c[:4]):
                    emit_chunk(m, zb, i == 0, False)
                proj(ta_r, True, False)
                for m in zb_c[4:]:
                    emit_chunk(m, zb, False, m == zb_c[-1])

                # tail: contract zb, finish proj, bias + store
                tb_r = work.tile([_R, _BL], bf16, tag="tb")
                k_contract(zb, tb_r, "zfb")
                proj(tb_r, False, True)

                osb = work.tile([_BL, _C], f32, tag="osb")
                nc.scalar.add(osb[:, :_SL], op0[:], bsb[:])
                nc.sync.dma_start(out=out_d[:, :_SL], in_=osb[:, :_SL])
                nc.scalar.add(osb[:, _SL:], op1[:], bsb[:])
                nc.sync.dma_start(out=out_d[:, _SL:], in_=osb[:, _SL:])

    _split_excess_waits(nc, mybir)
    return nc


def _get_program():
    if "nc" not in _cache:
        _cache["nc"] = _build_program()
    return _cache["nc"]


def _host_prep(x, weight, f0, f1, f2, f3, bias):
    """Shard x over cores (batch dim) in a DMA-friendly layout and cast
    to bf16; index-expand f0/f1 into the same ij = 128*m+p layout
    (gather + transpose + dtype cast only -- no arithmetic)."""
    import ml_dtypes

    bfl = ml_dtypes.bfloat16
    x = np.ascontiguousarray(np.asarray(x, dtype=np.float32))
    ij = np.arange(_IJ)
    f01 = np.stack(
        [np.asarray(f0, np.float32)[ij // _M2],
         np.asarray(f1, np.float32)[ij % _M2]], axis=1
    )  # [ij, 2, R]
    f01x = np.ascontiguousarray(
        f01.reshape(_NCH, 128, 2, _R).transpose(1, 0, 2, 3).astype(bfl)
    )
    f2t = np.ascontiguousarray(np.asarray(f2, np.float32).T)
    f3t = np.ascontiguousarray(np.asarray(f3, np.float32).T)
    w = np.ascontiguousarray(np.asarray(weight, np.float32).reshape(_R, 1))
    b = np.ascontiguousarray(np.asarray(bias, np.float32).reshape(1, 1))
    in_maps = []
    for c in range(_NCORES):
        xc = x[c * _BL : (c + 1) * _BL]
        # [b, ij, k] -> [p, m, b, k] with ij = 128*m + p
        xd = np.ascontiguousarray(
            xc.reshape(_BL, _NCH, 128, _M3)
            .transpose(2, 1, 0, 3)
            .astype(bfl)
        )
        in_maps.append(
            {"x": xd, "f01": f01x, "f2t": f2t, "f3t": f3t, "w": w, "b": b}
        )
    return in_maps


LAST_EXEC_NS = None


def kernel(x, weight, f0, f1, f2, f3, bias):
    global LAST_EXEC_NS
    from concourse.bass_utils import run_bass_kernel_spmd

    nc = _get_program()
    in_maps = _host_prep(x, weight, f0, f1, f2, f3, bias)
    trace = bool(int(os.environ.get("BASS_KERNEL_TRACE", "0")))
    res = run_bass_kernel_spmd(nc, in_maps, list(range(_NCORES)), trace=trace)
    LAST_EXEC_NS = res.exec_time_ns
    out = np.concatenate([res.results[c]["out"] for c in range(_NCORES)], axis=0)
    return np.ascontiguousarray(out.astype(np.float32, copy=False))
